# revision 15
# baseline (speedup 1.0000x reference)
"""Trainium2 Bass kernel for nn_LookaheadModel (B=16, T=2048, H=512, V=50257).

Strategy (8 NeuronCores, SPMD):
- Data-parallel over batch: core c owns batches [2c, 2c+1] for the encoder /
  selection / attention pipeline.
- Tensor-parallel over vocab for the output projection: core c computes
  logits[:, c*VS:(c+1)*VS] for ALL 16 batches after an AllGather of the
  16 context vectors.
- The reference's topk/gather/pad logic is reformulated mask-wise (exactly
  equivalent: attention is permutation-invariant over memory slots and
  MEM_SLOTS == K + R exactly, so only the selected SET matters):
    * fwd top-512 / retro top-128 become per-batch score thresholds found by
      a 4-round 128-way histogram search on device (final step 6e-8 <<
      min boundary gap ~4e-5 for this model, host-validated).
    * attention runs over all 2045 candidates with -1e9 added to unselected
      positions (exp underflows to exactly 0, matching the reference).
- The embedding gather runs on the host (indirect DMA unavailable here);
  the device receives pre-gathered transposed embeddings h0T per core.
- Heavy matmuls use float32r (~1.4e-4 rel err; end-to-end impact ~8e-4,
  validated against the reference including selection-flip effects).

Layouts: h-major everywhere ([h partitions, t free]); zero transposes.
Windowed context sums are PSUM-accumulated identity matmuls with clamped
widths (the ragged tail falls out of partial-width accumulation).
"""
import numpy as np
from contextlib import ExitStack

import concourse.bass as bass
import concourse.bacc as bacc
import concourse.tile as tile
from concourse import mybir
from concourse.bass_utils import run_bass_kernel_spmd

F32 = mybir.dt.float32
F32R = mybir.dt.float32r
BF16 = mybir.dt.bfloat16
AL = mybir.AluOpType
AF = mybir.ActivationFunctionType
AX = mybir.AxisListType

B, T, H, V = 16, 2048, 512, 50257
NC_ = T - 3              # 2045 candidates
K_FWD, K_RETRO = 512, 128
WINDOW = 8
NCORES = 8
BL = B // NCORES         # batches per core (2)
VS = 6284                # vocab shard (6284*8 = 50272 >= V, even chunks)
NCHUNK, CH = 4, 512      # encoder token chunks
EPS = 1e-5
HH = H // 128            # 4 h-tiles
JJ = 2 * H // 128        # 8 j-tiles
OC = 256                 # output-projection vocab chunk

SEARCH_LO = -16.0
SEARCH_STEPS = []
_step = 32.0 / 127.0
for _ in range(4):
    SEARCH_STEPS.append(_step)
    _step /= 126.0


def _build_nc():
    nc = bacc.Bacc(None, target_bir_lowering=False)
    D = lambda n, s, dt=F32: nc.declare_dram_parameter(n, s, dt, isOutput=False)

    h0T_d = D("h0T", [BL, H, T])
    ff1_d = D("ff1_w", [H, 2 * H]); ff1b_d = D("ff1_b", [2 * H])
    ff2_d = D("ff2_w", [2 * H, H]); ff2b_d = D("ff2_b", [H])
    lnw_d = D("ln_w", [H]); lnb_d = D("ln_b", [H])
    fg_d = D("fg_w", [H]); fgb_d = D("fg_b", [1])
    rg1_d = D("rg1_w", [2 * H, H]); rg1b_d = D("rg1_b", [H])
    rg2_d = D("rg2_w", [H]); rg2b_d = D("rg2_b", [1])
    qw_d = D("q_w", [H, H]); qb_d = D("q_b", [H])
    outw_d = D("out_w_sh", [H, VS]); outb_d = D("out_b_sh", [VS])
    recip_d = D("recip", [T])
    iota_d = D("iota", [128, 1])
    ident_d = D("ident", [128, 128])
    logits_d = nc.declare_dram_parameter("logits", [B, VS], F32, isOutput=True)

    with tile.TileContext(nc) as tc, ExitStack() as ctx:
        wpool = ctx.enter_context(tc.tile_pool(name="w", bufs=1))
        hpool = ctx.enter_context(tc.tile_pool(name="h", bufs=1))
        pool = ctx.enter_context(tc.tile_pool(name="p", bufs=1))
        pool2 = ctx.enter_context(tc.tile_pool(name="p2", bufs=2))
        rows = ctx.enter_context(tc.tile_pool(name="r", bufs=4))
        psumB = ctx.enter_context(tc.tile_pool(name="psB", bufs=1, space="PSUM"))
        psumA = ctx.enter_context(tc.tile_pool(name="psA", bufs=2, space="PSUM"))
        psumS = ctx.enter_context(tc.tile_pool(name="psS", bufs=2, space="PSUM"))
        dram = ctx.enter_context(tc.tile_pool(name="dr", bufs=1, space="DRAM"))

        def big_ps():
            return psumB.tile([128, T], F32, tag="big", name="bigps")

        # ---- persistent weights / constants ----
        ff1_sb = wpool.tile([128, HH, 2 * H], F32R)
        nc.sync.dma_start(ff1_sb[:], ff1_d[:, :].rearrange("(hh p) j -> p hh j", p=128).bitcast(F32R))
        ff2_sb = wpool.tile([128, JJ, H], F32R)
        nc.sync.dma_start(ff2_sb[:], ff2_d[:, :].rearrange("(jj p) h -> p jj h", p=128).bitcast(F32R))
        rg1_sb = wpool.tile([128, JJ, H], F32R)
        nc.sync.dma_start(rg1_sb[:], rg1_d[:, :].rearrange("(jj p) h -> p jj h", p=128).bitcast(F32R))
        qw_sb = wpool.tile([128, HH, H], F32R)
        nc.sync.dma_start(qw_sb[:], qw_d[:, :].rearrange("(hh p) j -> p hh j", p=128).bitcast(F32R))
        ident_sb = wpool.tile([128, 128], F32R)
        nc.sync.dma_start(ident_sb[:], ident_d[:, :].bitcast(F32R))
        fg_sb = wpool.tile([128, HH], F32R)
        nc.sync.dma_start(fg_sb[:], fg_d[:].rearrange("(hh p) -> p hh", p=128).bitcast(F32R))
        rg2_sb = wpool.tile([128, HH], F32R)
        nc.sync.dma_start(rg2_sb[:], rg2_d[:].rearrange("(hh p) -> p hh", p=128).bitcast(F32R))
        ff1b_sb = wpool.tile([128, JJ], F32)
        nc.sync.dma_start(ff1b_sb[:], ff1b_d[:].rearrange("(jj p) -> p jj", p=128))
        ff2b_sb = wpool.tile([128, HH], F32)
        nc.sync.dma_start(ff2b_sb[:], ff2b_d[:].rearrange("(hh p) -> p hh", p=128))
        rg1b_sb = wpool.tile([128, HH], F32)
        nc.sync.dma_start(rg1b_sb[:], rg1b_d[:].rearrange("(hh p) -> p hh", p=128))
        qb_sb = wpool.tile([128, HH], F32)
        nc.sync.dma_start(qb_sb[:], qb_d[:].rearrange("(hh p) -> p hh", p=128))
        lnw_sb = wpool.tile([128, HH], F32)
        nc.sync.dma_start(lnw_sb[:], lnw_d[:].rearrange("(hh p) -> p hh", p=128))
        lnb_sb = wpool.tile([128, HH], F32)
        nc.sync.dma_start(lnb_sb[:], lnb_d[:].rearrange("(hh p) -> p hh", p=128))
        fgb_sb = wpool.tile([1, 1], F32)
        nc.sync.dma_start(fgb_sb[:], fgb_d[None, :])
        rg2b_sb = wpool.tile([1, 1], F32)
        nc.sync.dma_start(rg2b_sb[:], rg2b_d[None, :])
        iota_sb = wpool.tile([128, 1], F32)
        nc.sync.dma_start(iota_sb[:], iota_d[:])
        eps_sb = wpool.tile([1, 1], F32)
        nc.vector.memset(eps_sb[:], EPS)
        ones_row = wpool.tile([1, 128], F32)
        nc.vector.memset(ones_row[:], 1.0)
        ones_colf = wpool.tile([128, 1], F32)
        nc.vector.memset(ones_colf[:], 1.0)
        ones_colr = wpool.tile([128, 1], F32R)
        nc.scalar.copy(ones_colr[:], ones_colf[:])
        ones16_f = wpool.tile([1, B], F32)
        nc.vector.memset(ones16_f[:], 1.0)
        ones16_r = wpool.tile([1, B], F32R)
        nc.scalar.copy(ones16_r[:], ones16_f[:])
        ctx_sb = wpool.tile([128, HH, BL], F32)
        zpad_f = wpool.tile([128, 16], F32)
        nc.vector.memset(zpad_f[:], 0.0)
        zpad_sb = wpool.tile([128, 16], F32R)
        nc.scalar.copy(zpad_sb[:], zpad_f[:])

        # recip replicated (fp32 exact broadcast)
        recip_row = rows.tile([1, T], F32, tag="row")
        nc.sync.dma_start(recip_row[:], recip_d[None, :])
        recrep_ps = big_ps()
        for q in range(T // 512):
            nc.tensor.matmul(recrep_ps[:, q * 512:(q + 1) * 512], ones_row[:],
                             recip_row[:, q * 512:(q + 1) * 512], start=True, stop=True)
        recip_rep = wpool.tile([128, T], F32)
        nc.scalar.copy(recip_rep[:], recrep_ps[:])

        def threshold_search(srow, k):
            """tau with count(srow > tau) == k. srow: [1, T] f32 sbuf row."""
            srep = pool.tile([128, T], F32, tag="srep")
            srep_ps = big_ps()
            for q in range(T // 512):
                nc.tensor.matmul(srep_ps[:, q * 512:(q + 1) * 512], ones_row[:],
                                 srow[:, q * 512:(q + 1) * 512], start=True, stop=True)
            nc.scalar.copy(srep[:], srep_ps[:])
            lo = None
            for r, step in enumerate(SEARCH_STEPS):
                tau_col = pool2.tile([128, 1], F32, tag="tcol")
                if r == 0:
                    nc.vector.tensor_scalar(tau_col[:], iota_sb[:], step, SEARCH_LO,
                                            AL.mult, AL.add)
                else:
                    lo_ps = psumS.tile([128, 1], F32, tag="small")
                    nc.tensor.matmul(lo_ps[:], ones_row[:], lo[:], start=True, stop=True)
                    nc.vector.scalar_tensor_tensor(tau_col[:], iota_sb[:], step,
                                                   lo_ps[:], AL.mult, AL.add)
                ntau = pool2.tile([128, 1], F32, tag="ntau")
                nc.vector.tensor_scalar_mul(ntau[:], tau_col[:], -1.0)
                sgn = pool.tile([128, T], BF16, tag="sgn")
                scnt = pool2.tile([128, 1], F32, tag="scnt")
                nc.scalar.activation(sgn[:], srep[:], AF.Sign, bias=ntau[:],
                                     scale=1.0, accum_out=scnt[:])
                g = pool2.tile([128, 1], F32, tag="g")
                nc.vector.tensor_scalar(g[:], scnt[:], float(2 * k - T) - 0.5, None,
                                        AL.is_ge)
                cnt_ps = psumS.tile([1, 1], F32, tag="small")
                nc.tensor.matmul(cnt_ps[:], g[:], ones_colf[:],
                                 start=True, stop=True)
                nlo = pool2.tile([1, 1], F32, tag="nlo")
                nc.vector.tensor_scalar(nlo[:], cnt_ps[:], -1.0, step, AL.add, AL.mult)
                if r == 0:
                    nc.vector.tensor_scalar_add(nlo[:], nlo[:], SEARCH_LO)
                else:
                    nc.vector.tensor_scalar_add(nlo[:], nlo[:], lo[:, 0:1])
                lo = nlo
            return lo

        for b in range(BL):
            hid = hpool.tile([128, HH, T], F32R, tag="hid")

            # ================= encoder =================
            for c in range(NCHUNK):
                t0 = c * CH
                h0 = pool2.tile([128, HH, CH], F32R, tag="h0hsb")
                nc.sync.dma_start(
                    h0[:], h0T_d[b, :, t0:t0 + CH].rearrange("(hh p) t -> p hh t", p=128).bitcast(F32R))

                hT_ps = big_ps().rearrange("p (hh t) -> p hh t", hh=HH)
                for jj in range(JJ):
                    y1_ps = psumS.tile([128, CH], F32, tag="small")
                    for hh in range(HH):
                        nc.tensor.matmul(y1_ps[:],
                                         ff1_sb[:, hh, jj * 128:(jj + 1) * 128],
                                         h0[:, hh, :],
                                         start=(hh == 0), stop=(hh == HH - 1))
                    y1 = pool2.tile([128, CH], F32R, tag="y1")
                    nc.scalar.activation(y1[:], y1_ps[:], AF.Relu,
                                         bias=ff1b_sb[:, jj:jj + 1], scale=1.0)
                    for hh in range(HH):
                        nc.tensor.matmul(hT_ps[:, hh, :],
                                         ff2_sb[:, jj, hh * 128:(hh + 1) * 128],
                                         y1[:],
                                         start=(jj == 0), stop=False)
                for hh in range(HH):  # residual
                    nc.tensor.matmul(hT_ps[:, hh, :], ident_sb[:], h0[:, hh, :],
                                     start=False, stop=True)

                hsb = pool2.tile([128, HH, CH], F32R, tag="h0hsb")
                mu_ps = psumA.tile([1, CH], F32, tag="acc")
                s2_ps = psumA.tile([1, CH], F32, tag="acc")
                for hh in range(HH):
                    nc.scalar.activation(hsb[:, hh, :], hT_ps[:, hh, :], AF.Identity,
                                         bias=ff2b_sb[:, hh:hh + 1], scale=1.0)
                    sq = pool2.tile([128, CH], F32R, tag="tmp512")
                    nc.scalar.activation(sq[:], hsb[:, hh, :], AF.Square)
                    nc.tensor.matmul(mu_ps[:], ones_colr[:], hsb[:, hh, :],
                                     start=(hh == 0), stop=(hh == HH - 1))
                    nc.tensor.matmul(s2_ps[:], ones_colr[:], sq[:],
                                     start=(hh == 0), stop=(hh == HH - 1))
                negmu = pool.tile([1, CH], F32, tag="negmu")
                nc.vector.tensor_scalar_mul(negmu[:], mu_ps[:], -1.0 / H)
                mu2 = pool.tile([1, CH], F32, tag="mu2")
                nc.scalar.activation(mu2[:], negmu[:], AF.Square)
                var_ps = psumS.tile([1, CH], F32, tag="small")
                nc.vector.scalar_tensor_tensor(var_ps[:], s2_ps[:], 1.0 / H, mu2[:],
                                               AL.mult, AL.subtract)
                sig_ps = psumS.tile([1, CH], F32, tag="small")
                nc.scalar.activation(sig_ps[:], var_ps[:], AF.Sqrt, bias=eps_sb[:], scale=1.0)
                rstd = pool.tile([1, CH], F32, tag="rstd")
                nc.vector.reciprocal(rstd[:], sig_ps[:])
                nm_ps = psumS.tile([128, CH], F32, tag="small")
                rs_ps = psumS.tile([128, CH], F32, tag="small")
                nc.tensor.matmul(nm_ps[:], ones_row[:], negmu[:], start=True, stop=True)
                nc.tensor.matmul(rs_ps[:], ones_row[:], rstd[:], start=True, stop=True)
                nmrs = pool.tile([128, 2, CH], F32, tag="nmrs")
                nc.scalar.copy(nmrs[:, 0, :], nm_ps[:])
                nc.scalar.copy(nmrs[:, 1, :], rs_ps[:])
                for hh in range(HH):
                    t1 = pool2.tile([128, CH], F32, tag="tmp512")
                    nc.vector.tensor_add(t1[:], hsb[:, hh, :], nmrs[:, 0, :])
                    nc.vector.tensor_mul(t1[:], t1[:], nmrs[:, 1, :])
                    nc.vector.tensor_scalar(hid[:, hh, t0:t0 + CH], t1[:],
                                            lnw_sb[:, hh:hh + 1], lnb_sb[:, hh:hh + 1],
                                            AL.mult, AL.add)

            # ================= fwd scores =================
            fwd_row = rows.tile([1, T], F32, tag="row")
            for c in range(NCHUNK):
                t0 = c * CH
                f_ps = psumA.tile([1, CH], F32, tag="acc")
                for hh in range(HH):
                    nc.tensor.matmul(f_ps[:], fg_sb[:, hh:hh + 1], hid[:, hh, t0:t0 + CH],
                                     start=(hh == 0), stop=(hh == HH - 1))
                nc.scalar.activation(fwd_row[:, t0:t0 + CH], f_ps[:], AF.Identity,
                                     bias=fgb_sb[:], scale=1.0)
            nc.vector.memset(fwd_row[:, NC_:T], -1.0e30)

            # ================= windowed ctx + retro logits =================
            retro_row = rows.tile([1, T], F32, tag="row")
            # zero-padded tail so the last chunk's window sums clamp at 2044
            # via genuinely-zero contributions (keeps fp32r widths even).
            htail = pool.tile([128, HH, CH + WINDOW], F32R, tag="htail")
            nvt = NC_ - (NCHUNK - 1) * CH    # 509: hid[1536:2045] are summable
            nc.vector.tensor_copy(htail[:, :, 0:nvt],
                                  hid[:, :, (NCHUNK - 1) * CH:NC_])
            for hh in range(HH):
                nc.vector.tensor_copy(htail[:, hh, nvt:CH + WINDOW],
                                      zpad_sb[:, 0:CH + WINDOW - nvt])
            for c in range(NCHUNK):
                t0 = c * CH
                seg_ps = big_ps().rearrange("p (hh t) -> p hh t", hh=HH)
                last = c == NCHUNK - 1
                for hh in range(HH):
                    for d in range(1, WINDOW + 1):
                        rhs = (hid[:, hh, t0 + d:t0 + d + CH] if not last
                               else htail[:, hh, d:d + CH])
                        nc.tensor.matmul(seg_ps[:, hh, :], ident_sb[:], rhs,
                                         start=(d == 1), stop=(d == WINDOW))
                ctxw = pool.tile([128, HH, CH], F32R, tag="ctxw")
                for hh in range(HH):
                    nc.vector.tensor_mul(ctxw[:, hh, :], seg_ps[:, hh, :],
                                         recip_rep[:, t0:t0 + CH])
                if last:
                    # position 2044: empty window -> ctx = hidden[2044]
                    nc.vector.tensor_copy(ctxw[:, :, NC_ - 1 - t0:NC_ - t0],
                                          hid[:, :, NC_ - 1:NC_])

                r_ps = psumA.tile([1, CH], F32, tag="acc")
                for hh in range(HH):
                    z_ps = psumS.tile([128, CH], F32, tag="small")
                    for kk in range(JJ):
                        rhs = hid[:, kk, t0:t0 + CH] if kk < HH else ctxw[:, kk - HH, :]
                        nc.tensor.matmul(z_ps[:],
                                         rg1_sb[:, kk, hh * 128:(hh + 1) * 128],
                                         rhs, start=(kk == 0), stop=(kk == JJ - 1))
                    z1 = pool2.tile([128, CH], F32R, tag="z1")
                    nc.scalar.activation(z1[:], z_ps[:], AF.Relu,
                                         bias=rg1b_sb[:, hh:hh + 1], scale=1.0)
                    nc.tensor.matmul(r_ps[:], rg2_sb[:, hh:hh + 1], z1[:],
                                     start=(hh == 0), stop=(hh == HH - 1))
                nc.scalar.activation(retro_row[:, t0:t0 + CH], r_ps[:], AF.Identity,
                                     bias=rg2b_sb[:], scale=1.0)
            nc.vector.memset(retro_row[:, NC_:T], -1.0e30)

            # ================= thresholds + masks =================
            tau_f = threshold_search(fwd_row, K_FWD)
            mf = rows.tile([1, T], F32, tag="row")
            nc.vector.tensor_scalar(mf[:], fwd_row[:], tau_f[:], None, AL.is_gt)
            rmask_row = rows.tile([1, T], F32, tag="row")
            nc.vector.scalar_tensor_tensor(rmask_row[:], mf[:], -1.0e30, retro_row[:],
                                           AL.mult, AL.add)
            tau_r = threshold_search(rmask_row, K_RETRO)
            mr = rows.tile([1, T], F32, tag="row")
            nc.vector.tensor_scalar(mr[:], rmask_row[:], tau_r[:], None, AL.is_gt)
            # mf, mr disjoint -> pen = mf + mr - 1 in {-1, 0}
            pen = rows.tile([1, T], F32, tag="row")
            nc.vector.scalar_tensor_tensor(pen[:], mf[:], -1.0, mr[:], AL.add, AL.add)

            # ================= q + attention =================
            q_ps = psumA.tile([128, HH, 2], F32, tag="acc")
            for mm in range(HH):
                for hh in range(HH):
                    nc.tensor.matmul(q_ps[:, mm, :],
                                     qw_sb[:, hh, mm * 128:(mm + 1) * 128],
                                     hid[:, hh, T - 2:T],
                                     start=(hh == 0), stop=(hh == HH - 1))
            q_sb = pool2.tile([128, HH], F32R, tag="qsb")
            for mm in range(HH):
                nc.scalar.activation(q_sb[:, mm:mm + 1], q_ps[:, mm, 0:1],
                                     AF.Identity, bias=qb_sb[:, mm:mm + 1], scale=1.0)

            att_row = rows.tile([1, T], F32, tag="row")
            for c in range(NCHUNK):
                t0 = c * CH
                a_ps = psumA.tile([1, CH], F32, tag="acc")
                for hh in range(HH):
                    nc.tensor.matmul(a_ps[:], q_sb[:, hh:hh + 1], hid[:, hh, t0:t0 + CH],
                                     start=(hh == 0), stop=(hh == HH - 1))
                nc.scalar.copy(att_row[:, t0:t0 + CH], a_ps[:])
            # att += pen * 1e9  (in-place elementwise on DVE)
            nc.vector.scalar_tensor_tensor(att_row[:], pen[:], 1.0e9, att_row[:],
                                           AL.mult, AL.add)

            amax = pool2.tile([1, 1], F32, tag="amax")
            nc.vector.tensor_reduce(amax[:], att_row[:], AX.X, AL.max)
            namax = pool2.tile([1, 1], F32, tag="namax")
            nc.vector.tensor_scalar_mul(namax[:], amax[:], -1.0)
            esum = pool2.tile([1, 1], F32, tag="esum")
            attn_row = rows.tile([1, T], F32, tag="row")
            nc.scalar.activation(attn_row[:], att_row[:], AF.Exp, bias=namax[:],
                                 scale=1.0, accum_out=esum[:])
            rsum = pool2.tile([1, 1], F32, tag="rsum")
            nc.vector.reciprocal(rsum[:], esum[:])

            # ctx = (sum_t e[t] * hid[:, t]) * (1/esum)
            at_ps = big_ps()
            for q in range(T // 512):
                nc.tensor.matmul(at_ps[:, q * 512:(q + 1) * 512], ones_row[:],
                                 attn_row[:, q * 512:(q + 1) * 512], start=True, stop=True)
            rs_col_ps = psumS.tile([128, 1], F32, tag="small")
            nc.tensor.matmul(rs_col_ps[:], ones_row[:], rsum[:], start=True, stop=True)
            rs_col = pool2.tile([128, 1], F32, tag="rscols")
            nc.scalar.copy(rs_col[:], rs_col_ps[:])
            for hh in range(HH):
                craw = pool2.tile([128, 4], F32, tag="craw")
                for q in range(T // 512):
                    prod = pool2.tile([128, CH], F32, tag="tmp512")
                    nc.vector.tensor_mul(prod[:], hid[:, hh, q * 512:(q + 1) * 512],
                                         at_ps[:, q * 512:(q + 1) * 512])
                    nc.vector.tensor_reduce(craw[:, q:q + 1], prod[:], AX.X, AL.add)
                craw2 = pool2.tile([128, 1], F32, tag="craw2")
                nc.vector.tensor_reduce(craw2[:], craw[:], AX.X, AL.add)
                nc.vector.tensor_scalar(ctx_sb[:, hh, b:b + 1], craw2[:], rs_col[:],
                                        None, AL.mult)

        # ================= allgather + output projection =================
        ctxl_d = dram.tile([BL, H], F32)
        for b in range(BL):
            nc.sync.dma_start(ctxl_d[b].rearrange("(hh p) -> p hh", p=128),
                              ctx_sb[:, :, b:b + 1].rearrange("p hh o -> p (hh o)"))
        ag_d = dram.tile([B, H], F32)
        nc.gpsimd.collective_compute(
            "AllGather", AL.bypass,
            replica_groups=[list(range(NCORES))],
            ins=[ctxl_d.opt()], outs=[ag_d.opt()])
        agT = wpool.tile([128, HH, B], F32R)
        for hh in range(HH):
            nc.sync.dma_start(
                agT[:, hh, :],
                ag_d[:, hh * 128:(hh + 1) * 128].rearrange("b p -> p b").bitcast(F32R))

        nv = 0
        while nv < VS:
            nn_ = min(OC, VS - nv)
            ow = pool.tile([128, HH, OC], F32R, tag="ow")
            nc.sync.dma_start(ow[:, :, 0:nn_],
                              outw_d[:, nv:nv + nn_].rearrange("(hh p) n -> p hh n", p=128).bitcast(F32R))
            ob = pool2.tile([1, OC], F32R, tag="ob")
            nc.sync.dma_start(ob[:, 0:nn_], outb_d[None, nv:nv + nn_].bitcast(F32R))
            l_ps = psumA.tile([B, OC], F32, tag="acc")
            for hh in range(HH):
                nc.tensor.matmul(l_ps[:, 0:nn_], agT[:, hh, :], ow[:, hh, 0:nn_],
                                 start=(hh == 0), stop=False)
            nc.tensor.matmul(l_ps[:, 0:nn_], ones16_r[:], ob[:, 0:nn_],
                             start=False, stop=True)
            lsb = pool2.tile([B, OC], F32, tag="lsb")
            nc.scalar.copy(lsb[:, 0:nn_], l_ps[:, 0:nn_])
            nc.sync.dma_start(logits_d[:, nv:nv + nn_], lsb[:, 0:nn_])
            nv += nn_

    nc.finalize()
    return nc


_NC_CACHE = {}


def _get_nc():
    if "nc" not in _NC_CACHE:
        _NC_CACHE["nc"] = _build_nc()
    return _NC_CACHE["nc"]


def _host_prep(inputs):
    seq = np.asarray(inputs["seq"]).astype(np.int64)
    embed = np.asarray(inputs["embed"], dtype=np.float32)
    embedT = np.ascontiguousarray(embed.T)

    recip = np.ones(T, np.float32)
    s = np.arange(NC_)
    cnt = np.minimum(s + 1 + WINDOW, NC_) - s - 1
    recip[:NC_] = 1.0 / np.maximum(cnt, 1)
    recip[NC_ - 1] = 1.0  # empty window -> override slot uses hidden directly

    outw = np.asarray(inputs["out_w"], dtype=np.float32)
    outb = np.asarray(inputs["out_b"], dtype=np.float32)
    outw_pad = np.zeros((H, VS * NCORES), np.float32)
    outw_pad[:, :V] = outw
    outb_pad = np.zeros(VS * NCORES, np.float32)
    outb_pad[:V] = outb

    shared = {
        "ff1_w": np.asarray(inputs["ff1_w"], np.float32),
        "ff1_b": np.asarray(inputs["ff1_b"], np.float32),
        "ff2_w": np.asarray(inputs["ff2_w"], np.float32),
        "ff2_b": np.asarray(inputs["ff2_b"], np.float32),
        "ln_w": np.asarray(inputs["ln_w"], np.float32),
        "ln_b": np.asarray(inputs["ln_b"], np.float32),
        "fg_w": np.asarray(inputs["fg_w"], np.float32),
        "fg_b": np.asarray(inputs["fg_b"], np.float32).reshape(1),
        "rg1_w": np.asarray(inputs["rg1_w"], np.float32),
        "rg1_b": np.asarray(inputs["rg1_b"], np.float32),
        "rg2_w": np.asarray(inputs["rg2_w"], np.float32),
        "rg2_b": np.asarray(inputs["rg2_b"], np.float32).reshape(1),
        "q_w": np.asarray(inputs["q_w"], np.float32),
        "q_b": np.asarray(inputs["q_b"], np.float32),
        "recip": recip,
        "iota": np.arange(128, dtype=np.float32).reshape(128, 1),
        "ident": np.eye(128, dtype=np.float32),
    }
    in_maps = []
    for c in range(NCORES):
        m = dict(shared)
        h0T = np.empty((BL, H, T), np.float32)
        for b in range(BL):
            h0T[b] = embedT[:, seq[c * BL + b]]
        m["h0T"] = h0T
        m["out_w_sh"] = np.ascontiguousarray(outw_pad[:, c * VS:(c + 1) * VS])
        m["out_b_sh"] = np.ascontiguousarray(outb_pad[c * VS:(c + 1) * VS])
        in_maps.append(m)
    return in_maps


def kernel(**inputs):
    in_maps = _host_prep(inputs)
    nc = _get_nc()
    res = run_bass_kernel_spmd(nc, in_maps, list(range(NCORES)))
    out = np.concatenate([res.results[c]["logits"] for c in range(NCORES)], axis=1)
    return np.ascontiguousarray(out[:, :V])


# revision 17
# speedup vs baseline: 1.0358x; 1.0358x over previous
"""Trainium2 Bass kernel for nn_LookaheadModel (B=16, T=2048, H=512, V=50257).

Strategy (8 NeuronCores, SPMD):
- Data-parallel over batch: core c owns batches [2c, 2c+1] for the encoder /
  selection / attention pipeline.
- Tensor-parallel over vocab for the output projection: core c computes
  logits[:, c*VS:(c+1)*VS] for ALL 16 batches after an AllGather of the
  16 context vectors.
- The reference's topk/gather/pad logic is reformulated mask-wise (exactly
  equivalent: attention is permutation-invariant over memory slots and
  MEM_SLOTS == K + R exactly, so only the selected SET matters):
    * fwd top-512 / retro top-128 become per-batch score thresholds found by
      a 4-round 128-way histogram search on device (final step 6e-8 <<
      min boundary gap ~4e-5 for this model, host-validated).
    * attention runs over all 2045 candidates with -1e9 added to unselected
      positions (exp underflows to exactly 0, matching the reference).
- The embedding gather runs on the host (indirect DMA unavailable here);
  the device receives pre-gathered transposed embeddings h0T per core.
- Heavy matmuls use float32r (~1.4e-4 rel err; end-to-end impact ~8e-4,
  validated against the reference including selection-flip effects).

Layouts: h-major everywhere ([h partitions, t free]); zero transposes.
Windowed context sums are PSUM-accumulated identity matmuls with clamped
widths (the ragged tail falls out of partial-width accumulation).
"""
import numpy as np
from contextlib import ExitStack

import concourse.bass as bass
import concourse.bacc as bacc
import concourse.tile as tile
from concourse import mybir
from concourse.bass_utils import run_bass_kernel_spmd

F32 = mybir.dt.float32
F32R = mybir.dt.float32r
BF16 = mybir.dt.bfloat16
AL = mybir.AluOpType
AF = mybir.ActivationFunctionType
AX = mybir.AxisListType

B, T, H, V = 16, 2048, 512, 50257
NC_ = T - 3              # 2045 candidates
K_FWD, K_RETRO = 512, 128
WINDOW = 8
NCORES = 8
BL = B // NCORES         # batches per core (2)
VS = 6284                # vocab shard (6284*8 = 50272 >= V, even chunks)
NCHUNK, CH = 4, 512      # encoder token chunks
EPS = 1e-5
HH = H // 128            # 4 h-tiles
JJ = 2 * H // 128        # 8 j-tiles
OC = 256                 # output-projection vocab chunk

SEARCH_LO = -16.0
SEARCH_STEPS = []
_step = 32.0 / 127.0
for _ in range(4):
    SEARCH_STEPS.append(_step)
    _step /= 126.0


def _build_nc():
    nc = bacc.Bacc(None, target_bir_lowering=False)
    D = lambda n, s, dt=F32: nc.declare_dram_parameter(n, s, dt, isOutput=False)

    h0T_d = D("h0T", [BL, H, T])
    ff1_d = D("ff1_w", [H, 2 * H]); ff1b_d = D("ff1_b", [2 * H])
    ff2_d = D("ff2_w", [2 * H, H]); ff2b_d = D("ff2_b", [H])
    lnw_d = D("ln_w", [H]); lnb_d = D("ln_b", [H])
    fg_d = D("fg_w", [H]); fgb_d = D("fg_b", [1])
    rg1_d = D("rg1_w", [2 * H, H]); rg1b_d = D("rg1_b", [H])
    rg2_d = D("rg2_w", [H]); rg2b_d = D("rg2_b", [1])
    qw_d = D("q_w", [H, H]); qb_d = D("q_b", [H])
    outw_d = D("out_w_sh", [H, VS]); outb_d = D("out_b_sh", [VS])
    recip_d = D("recip", [T])
    iota_d = D("iota", [128, 1])
    ident_d = D("ident", [128, 128])
    logits_d = nc.declare_dram_parameter("logits", [B, VS], F32, isOutput=True)

    with tile.TileContext(nc) as tc, ExitStack() as ctx:
        wpool = ctx.enter_context(tc.tile_pool(name="w", bufs=1))
        hpool = ctx.enter_context(tc.tile_pool(name="h", bufs=1))
        pool = ctx.enter_context(tc.tile_pool(name="p", bufs=1))
        pool2 = ctx.enter_context(tc.tile_pool(name="p2", bufs=2))
        rows = ctx.enter_context(tc.tile_pool(name="r", bufs=4))
        psumB = ctx.enter_context(tc.tile_pool(name="psB", bufs=1, space="PSUM"))
        psumA = ctx.enter_context(tc.tile_pool(name="psA", bufs=2, space="PSUM"))
        psumS = ctx.enter_context(tc.tile_pool(name="psS", bufs=2, space="PSUM"))
        dram = ctx.enter_context(tc.tile_pool(name="dr", bufs=1, space="DRAM"))

        def big_ps():
            return psumB.tile([128, T], F32, tag="big", name="bigps")

        # ---- persistent weights / constants ----
        ff1_sb = wpool.tile([128, HH, 2 * H], F32R)
        nc.sync.dma_start(ff1_sb[:], ff1_d[:, :].rearrange("(hh p) j -> p hh j", p=128).bitcast(F32R))
        ff2_sb = wpool.tile([128, JJ, H], F32R)
        nc.sync.dma_start(ff2_sb[:], ff2_d[:, :].rearrange("(jj p) h -> p jj h", p=128).bitcast(F32R))
        rg1_sb = wpool.tile([128, JJ, H], F32R)
        nc.sync.dma_start(rg1_sb[:], rg1_d[:, :].rearrange("(jj p) h -> p jj h", p=128).bitcast(F32R))
        qw_sb = wpool.tile([128, HH, H], F32R)
        nc.sync.dma_start(qw_sb[:], qw_d[:, :].rearrange("(hh p) j -> p hh j", p=128).bitcast(F32R))
        ident_sb = wpool.tile([128, 128], F32R)
        nc.sync.dma_start(ident_sb[:], ident_d[:, :].bitcast(F32R))
        fg_sb = wpool.tile([128, HH], F32R)
        nc.sync.dma_start(fg_sb[:], fg_d[:].rearrange("(hh p) -> p hh", p=128).bitcast(F32R))
        rg2_sb = wpool.tile([128, HH], F32R)
        nc.sync.dma_start(rg2_sb[:], rg2_d[:].rearrange("(hh p) -> p hh", p=128).bitcast(F32R))
        ff1b_sb = wpool.tile([128, JJ], F32)
        nc.sync.dma_start(ff1b_sb[:], ff1b_d[:].rearrange("(jj p) -> p jj", p=128))
        ff2b_sb = wpool.tile([128, HH], F32)
        nc.sync.dma_start(ff2b_sb[:], ff2b_d[:].rearrange("(hh p) -> p hh", p=128))
        rg1b_sb = wpool.tile([128, HH], F32)
        nc.sync.dma_start(rg1b_sb[:], rg1b_d[:].rearrange("(hh p) -> p hh", p=128))
        qb_sb = wpool.tile([128, HH], F32)
        nc.sync.dma_start(qb_sb[:], qb_d[:].rearrange("(hh p) -> p hh", p=128))
        lnw_sb = wpool.tile([128, HH], F32)
        nc.sync.dma_start(lnw_sb[:], lnw_d[:].rearrange("(hh p) -> p hh", p=128))
        lnb_sb = wpool.tile([128, HH], F32)
        nc.sync.dma_start(lnb_sb[:], lnb_d[:].rearrange("(hh p) -> p hh", p=128))
        fgb_sb = wpool.tile([1, 1], F32)
        nc.sync.dma_start(fgb_sb[:], fgb_d[None, :])
        rg2b_sb = wpool.tile([1, 1], F32)
        nc.sync.dma_start(rg2b_sb[:], rg2b_d[None, :])
        iota_sb = wpool.tile([128, 1], F32)
        nc.sync.dma_start(iota_sb[:], iota_d[:])
        eps_sb = wpool.tile([1, 1], F32)
        nc.vector.memset(eps_sb[:], EPS)
        ones_row = wpool.tile([1, 128], F32)
        nc.vector.memset(ones_row[:], 1.0)
        ones_colf = wpool.tile([128, 1], F32)
        nc.vector.memset(ones_colf[:], 1.0)
        ones_colr = wpool.tile([128, 1], F32R)
        nc.scalar.copy(ones_colr[:], ones_colf[:])
        ones16_f = wpool.tile([1, B], F32)
        nc.vector.memset(ones16_f[:], 1.0)
        ones16_r = wpool.tile([1, B], F32R)
        nc.scalar.copy(ones16_r[:], ones16_f[:])
        ctx_sb = wpool.tile([128, HH, BL], F32)
        zpad_f = wpool.tile([128, 32], F32)
        nc.vector.memset(zpad_f[:], 0.0)
        zpad_sb = wpool.tile([128, 32], F32R)
        nc.scalar.copy(zpad_sb[:], zpad_f[:])

        # recip replicated (fp32 exact broadcast)
        recip_row = rows.tile([1, T], F32, tag="row")
        nc.sync.dma_start(recip_row[:], recip_d[None, :])
        recrep_ps = big_ps()
        for q in range(T // 512):
            nc.tensor.matmul(recrep_ps[:, q * 512:(q + 1) * 512], ones_row[:],
                             recip_row[:, q * 512:(q + 1) * 512], start=True, stop=True)
        recip_rep = wpool.tile([128, T], F32)
        nc.scalar.copy(recip_rep[:], recrep_ps[:])

        def threshold_search(srow, k):
            """tau with count(srow > tau) == k. srow: [1, T] f32 sbuf row."""
            srep = pool.tile([128, T], F32, tag="srep")
            srep_ps = big_ps()
            for q in range(T // 512):
                nc.tensor.matmul(srep_ps[:, q * 512:(q + 1) * 512], ones_row[:],
                                 srow[:, q * 512:(q + 1) * 512], start=True, stop=True)
            nc.scalar.copy(srep[:], srep_ps[:])
            lo = None
            for r, step in enumerate(SEARCH_STEPS):
                tau_col = pool2.tile([128, 1], F32, tag="tcol")
                if r == 0:
                    nc.vector.tensor_scalar(tau_col[:], iota_sb[:], step, SEARCH_LO,
                                            AL.mult, AL.add)
                else:
                    lo_ps = psumS.tile([128, 1], F32, tag="small")
                    nc.tensor.matmul(lo_ps[:], ones_row[:], lo[:], start=True, stop=True)
                    nc.vector.scalar_tensor_tensor(tau_col[:], iota_sb[:], step,
                                                   lo_ps[:], AL.mult, AL.add)
                ntau = pool2.tile([128, 1], F32, tag="ntau")
                nc.vector.tensor_scalar_mul(ntau[:], tau_col[:], -1.0)
                sgn = pool.tile([128, T], BF16, tag="sgn")
                scnt = pool2.tile([128, 1], F32, tag="scnt")
                nc.scalar.activation(sgn[:], srep[:], AF.Sign, bias=ntau[:],
                                     scale=1.0, accum_out=scnt[:])
                g = pool2.tile([128, 1], F32, tag="g")
                nc.vector.tensor_scalar(g[:], scnt[:], float(2 * k - T) - 0.5, None,
                                        AL.is_ge)
                cnt_ps = psumS.tile([1, 1], F32, tag="small")
                nc.tensor.matmul(cnt_ps[:], g[:], ones_colf[:],
                                 start=True, stop=True)
                nlo = pool2.tile([1, 1], F32, tag="nlo")
                nc.vector.tensor_scalar(nlo[:], cnt_ps[:], -1.0, step, AL.add, AL.mult)
                if r == 0:
                    nc.vector.tensor_scalar_add(nlo[:], nlo[:], SEARCH_LO)
                else:
                    nc.vector.tensor_scalar_add(nlo[:], nlo[:], lo[:, 0:1])
                lo = nlo
            return lo

        for b in range(BL):
            hid = hpool.tile([128, HH, T], F32R, tag="hid")

            def enc_matmul_phase(c):
                t0 = c * CH
                h0 = pool2.tile([128, HH, CH], F32R, tag="h0hsb", name="h0")
                nc.sync.dma_start(
                    h0[:], h0T_d[b, :, t0:t0 + CH].rearrange("(hh p) t -> p hh t", p=128).bitcast(F32R))
                hT_ps = big_ps().rearrange("p (hh t) -> p hh t", hh=HH)
                for jj in range(JJ):
                    y1_ps = psumS.tile([128, CH], F32, tag="small", name="y1ps")
                    for hh in range(HH):
                        nc.tensor.matmul(y1_ps[:],
                                         ff1_sb[:, hh, jj * 128:(jj + 1) * 128],
                                         h0[:, hh, :],
                                         start=(hh == 0), stop=(hh == HH - 1))
                    y1 = pool2.tile([128, CH], F32R, tag="y1", name="y1")
                    nc.scalar.activation(y1[:], y1_ps[:], AF.Relu,
                                         bias=ff1b_sb[:, jj:jj + 1], scale=1.0)
                    for hh in range(HH):
                        nc.tensor.matmul(hT_ps[:, hh, :],
                                         ff2_sb[:, jj, hh * 128:(hh + 1) * 128],
                                         y1[:],
                                         start=(jj == 0), stop=False)
                for hh in range(HH):  # residual
                    nc.tensor.matmul(hT_ps[:, hh, :], ident_sb[:], h0[:, hh, :],
                                     start=False, stop=True)
                return hT_ps

            def enc_ln_phase(c, hT_ps):
                t0 = c * CH
                hsb = pool2.tile([128, HH, CH], F32R, tag="h0hsb", name="hsb")
                mu_ps = psumA.tile([1, CH], F32, tag="acc", name="mups")
                s2_ps = psumA.tile([1, CH], F32, tag="acc", name="s2ps")
                for hh in range(HH):
                    nc.scalar.activation(hsb[:, hh, :], hT_ps[:, hh, :], AF.Identity,
                                         bias=ff2b_sb[:, hh:hh + 1], scale=1.0)
                    sq = pool2.tile([128, CH], F32R, tag="tmp512", name="sq")
                    nc.scalar.activation(sq[:], hsb[:, hh, :], AF.Square)
                    nc.tensor.matmul(mu_ps[:], ones_colr[:], hsb[:, hh, :],
                                     start=(hh == 0), stop=(hh == HH - 1))
                    nc.tensor.matmul(s2_ps[:], ones_colr[:], sq[:],
                                     start=(hh == 0), stop=(hh == HH - 1))
                negmu = pool.tile([1, CH], F32, tag="negmu", name="negmu")
                nc.vector.tensor_scalar_mul(negmu[:], mu_ps[:], -1.0 / H)
                mu2 = pool.tile([1, CH], F32, tag="mu2", name="mu2")
                nc.scalar.activation(mu2[:], negmu[:], AF.Square)
                var_ps = psumS.tile([1, CH], F32, tag="small", name="varps")
                nc.vector.scalar_tensor_tensor(var_ps[:], s2_ps[:], 1.0 / H, mu2[:],
                                               AL.mult, AL.subtract)
                # rstd = exp(-0.5 * ln(var + eps))
                lnv = pool.tile([1, CH], F32, tag="lnv", name="lnv")
                nc.scalar.activation(lnv[:], var_ps[:], AF.Ln, bias=eps_sb[:], scale=1.0)
                rstd = pool.tile([1, CH], F32, tag="rstd", name="rstd")
                nc.scalar.activation(rstd[:], lnv[:], AF.Exp, scale=-0.5)
                nm_ps = psumS.tile([128, CH], F32, tag="small", name="nmps")
                rs_ps = psumS.tile([128, CH], F32, tag="small", name="rsps")
                nc.tensor.matmul(nm_ps[:], ones_row[:], negmu[:], start=True, stop=True)
                nc.tensor.matmul(rs_ps[:], ones_row[:], rstd[:], start=True, stop=True)
                for hh in range(HH):
                    t1 = pool2.tile([128, CH], F32, tag="tmp512", name="t1")
                    nc.vector.tensor_add(t1[:], hsb[:, hh, :], nm_ps[:])
                    nc.vector.tensor_mul(t1[:], t1[:], rs_ps[:])
                    nc.vector.tensor_scalar(hid[:, hh, t0:t0 + CH], t1[:],
                                            lnw_sb[:, hh:hh + 1], lnb_sb[:, hh:hh + 1],
                                            AL.mult, AL.add)

            def fwd_score_phase(c, fwd_row):
                t0 = c * CH
                f_ps = psumA.tile([1, CH], F32, tag="acc", name="fps")
                for hh in range(HH):
                    nc.tensor.matmul(f_ps[:], fg_sb[:, hh:hh + 1], hid[:, hh, t0:t0 + CH],
                                     start=(hh == 0), stop=(hh == HH - 1))
                nc.scalar.activation(fwd_row[:, t0:t0 + CH], f_ps[:], AF.Identity,
                                     bias=fgb_sb[:], scale=1.0)

            def make_htail():
                htail = pool.tile([128, HH, CH + 16], F32R, tag="htail", name="htail")
                nvt = NC_ - (NCHUNK - 1) * CH    # 509: hid[1536:2045] are summable
                nc.vector.tensor_copy(htail[:, :, 0:nvt],
                                      hid[:, :, (NCHUNK - 1) * CH:NC_])
                for hh in range(HH):
                    nc.vector.tensor_copy(htail[:, hh, nvt:CH + 16],
                                          zpad_sb[:, 0:CH + 16 - nvt])
                return htail

            def retro_phase(c, retro_row, htail):
                t0 = c * CH
                last = c == NCHUNK - 1
                ctxw = pool.tile([128, HH, CH], F32R, tag="ctxw", name="ctxw")
                for hh in range(HH):
                    src_ = htail[:, hh, :] if last else hid[:, hh, t0:t0 + CH + 16]
                    w2 = pool.tile([128, CH + 10], F32, tag="w2", name="w2")
                    nc.vector.tensor_add(w2[:], src_[:, 1:CH + 11], src_[:, 2:CH + 12])
                    w4 = pool.tile([128, CH + 6], F32, tag="w4", name="w4")
                    nc.vector.tensor_add(w4[:], w2[:, 0:CH + 6], w2[:, 2:CH + 8])
                    seg = pool2.tile([128, CH], F32, tag="tmp512", name="seg")
                    nc.vector.tensor_add(seg[:], w4[:, 0:CH], w4[:, 4:CH + 4])
                    nc.vector.tensor_mul(ctxw[:, hh, :], seg[:],
                                         recip_rep[:, t0:t0 + CH])
                if last:
                    # position 2044: empty window -> ctx = hidden[2044]
                    nc.vector.tensor_copy(ctxw[:, :, NC_ - 1 - t0:NC_ - t0],
                                          hid[:, :, NC_ - 1:NC_])
                r_ps = psumA.tile([1, CH], F32, tag="acc", name="rps")
                for hh in range(HH):
                    z_ps = psumS.tile([128, CH], F32, tag="small", name="zps")
                    for kk in range(JJ):
                        rhs = hid[:, kk, t0:t0 + CH] if kk < HH else ctxw[:, kk - HH, :]
                        nc.tensor.matmul(z_ps[:],
                                         rg1_sb[:, kk, hh * 128:(hh + 1) * 128],
                                         rhs, start=(kk == 0), stop=(kk == JJ - 1))
                    z1 = pool2.tile([128, CH], F32R, tag="z1", name="z1")
                    nc.scalar.activation(z1[:], z_ps[:], AF.Relu,
                                         bias=rg1b_sb[:, hh:hh + 1], scale=1.0)
                    nc.tensor.matmul(r_ps[:], rg2_sb[:, hh:hh + 1], z1[:],
                                     start=(hh == 0), stop=(hh == HH - 1))
                nc.scalar.activation(retro_row[:, t0:t0 + CH], r_ps[:], AF.Identity,
                                     bias=rg2b_sb[:], scale=1.0)

            # ---- software-pipelined emission ----
            fwd_row = rows.tile([1, T], F32, tag="row")
            retro_row = rows.tile([1, T], F32, tag="row")
            hT0 = enc_matmul_phase(0)
            hT1 = enc_matmul_phase(1)
            enc_ln_phase(0, hT0)
            hT2 = enc_matmul_phase(2)
            enc_ln_phase(1, hT1)
            fwd_score_phase(0, fwd_row)
            hT3 = enc_matmul_phase(3)
            enc_ln_phase(2, hT2)
            fwd_score_phase(1, fwd_row)
            retro_phase(0, retro_row, None)
            enc_ln_phase(3, hT3)
            fwd_score_phase(2, fwd_row)
            fwd_score_phase(3, fwd_row)
            nc.vector.memset(fwd_row[:, NC_:T], -1.0e30)
            retro_phase(1, retro_row, None)
            tau_f = threshold_search(fwd_row, K_FWD)
            retro_phase(2, retro_row, None)
            htail = make_htail()
            retro_phase(3, retro_row, htail)
            nc.vector.memset(retro_row[:, NC_:T], -1.0e30)

            # ================= masks =================
            mf = rows.tile([1, T], F32, tag="row")
            nc.vector.tensor_scalar(mf[:], fwd_row[:], tau_f[:], None, AL.is_gt)
            rmask_row = rows.tile([1, T], F32, tag="row")
            nc.vector.scalar_tensor_tensor(rmask_row[:], mf[:], -1.0e30, retro_row[:],
                                           AL.mult, AL.add)
            tau_r = threshold_search(rmask_row, K_RETRO)
            mr = rows.tile([1, T], F32, tag="row")
            nc.vector.tensor_scalar(mr[:], rmask_row[:], tau_r[:], None, AL.is_gt)
            # mf, mr disjoint -> pen = mf + mr - 1 in {-1, 0}
            pen = rows.tile([1, T], F32, tag="row")
            nc.vector.scalar_tensor_tensor(pen[:], mf[:], -1.0, mr[:], AL.add, AL.add)

            # ================= q + attention =================
            q_ps = psumA.tile([128, HH, 2], F32, tag="acc")
            for mm in range(HH):
                for hh in range(HH):
                    nc.tensor.matmul(q_ps[:, mm, :],
                                     qw_sb[:, hh, mm * 128:(mm + 1) * 128],
                                     hid[:, hh, T - 2:T],
                                     start=(hh == 0), stop=(hh == HH - 1))
            q_sb = pool2.tile([128, HH], F32R, tag="qsb")
            for mm in range(HH):
                nc.scalar.activation(q_sb[:, mm:mm + 1], q_ps[:, mm, 0:1],
                                     AF.Identity, bias=qb_sb[:, mm:mm + 1], scale=1.0)

            att_row = rows.tile([1, T], F32, tag="row")
            for c in range(NCHUNK):
                t0 = c * CH
                a_ps = psumA.tile([1, CH], F32, tag="acc", name="aps")
                for hh in range(HH):
                    nc.tensor.matmul(a_ps[:], q_sb[:, hh:hh + 1], hid[:, hh, t0:t0 + CH],
                                     start=(hh == 0), stop=(hh == HH - 1))
                nc.scalar.copy(att_row[:, t0:t0 + CH], a_ps[:])
            # att += pen * 1e9  (in-place elementwise on DVE)
            nc.vector.scalar_tensor_tensor(att_row[:], pen[:], 1.0e9, att_row[:],
                                           AL.mult, AL.add)

            amax = pool2.tile([1, 1], F32, tag="amax")
            nc.vector.tensor_reduce(amax[:], att_row[:], AX.X, AL.max)
            namax = pool2.tile([1, 1], F32, tag="namax")
            nc.vector.tensor_scalar_mul(namax[:], amax[:], -1.0)
            esum = pool2.tile([1, 1], F32, tag="esum")
            attn_row = rows.tile([1, T], F32, tag="row")
            nc.scalar.activation(attn_row[:], att_row[:], AF.Exp, bias=namax[:],
                                 scale=1.0, accum_out=esum[:])
            rsum = pool2.tile([1, 1], F32, tag="rsum")
            nc.vector.reciprocal(rsum[:], esum[:])

            # ctx = (sum_t e[t] * hid[:, t]) * (1/esum)
            at_ps = big_ps()
            for q in range(T // 512):
                nc.tensor.matmul(at_ps[:, q * 512:(q + 1) * 512], ones_row[:],
                                 attn_row[:, q * 512:(q + 1) * 512], start=True, stop=True)
            rs_col_ps = psumS.tile([128, 1], F32, tag="small")
            nc.tensor.matmul(rs_col_ps[:], ones_row[:], rsum[:], start=True, stop=True)
            rs_col = pool2.tile([128, 1], F32, tag="rscols")
            nc.scalar.copy(rs_col[:], rs_col_ps[:])
            for hh in range(HH):
                craw = pool2.tile([128, 4], F32, tag="craw")
                for q in range(T // 512):
                    prod = pool2.tile([128, CH], F32, tag="tmp512", name="prod")
                    nc.vector.tensor_mul(prod[:], hid[:, hh, q * 512:(q + 1) * 512],
                                         at_ps[:, q * 512:(q + 1) * 512])
                    nc.vector.tensor_reduce(craw[:, q:q + 1], prod[:], AX.X, AL.add)
                craw2 = pool2.tile([128, 1], F32, tag="craw2")
                nc.vector.tensor_reduce(craw2[:], craw[:], AX.X, AL.add)
                nc.vector.tensor_scalar(ctx_sb[:, hh, b:b + 1], craw2[:], rs_col[:],
                                        None, AL.mult)

        # ================= allgather + output projection =================
        ctxl_d = dram.tile([BL, H], F32)
        for b in range(BL):
            nc.sync.dma_start(ctxl_d[b].rearrange("(hh p) -> p hh", p=128),
                              ctx_sb[:, :, b:b + 1].rearrange("p hh o -> p (hh o)"))
        ag_d = dram.tile([B, H], F32)
        nc.gpsimd.collective_compute(
            "AllGather", AL.bypass,
            replica_groups=[list(range(NCORES))],
            ins=[ctxl_d.opt()], outs=[ag_d.opt()])
        agT = wpool.tile([128, HH, B], F32R)
        for hh in range(HH):
            nc.sync.dma_start(
                agT[:, hh, :],
                ag_d[:, hh * 128:(hh + 1) * 128].rearrange("b p -> p b").bitcast(F32R))

        nv = 0
        while nv < VS:
            nn_ = min(OC, VS - nv)
            ow = pool.tile([128, HH, OC], F32R, tag="ow")
            nc.sync.dma_start(ow[:, :, 0:nn_],
                              outw_d[:, nv:nv + nn_].rearrange("(hh p) n -> p hh n", p=128).bitcast(F32R))
            ob = pool2.tile([1, OC], F32R, tag="ob")
            nc.sync.dma_start(ob[:, 0:nn_], outb_d[None, nv:nv + nn_].bitcast(F32R))
            l_ps = psumA.tile([B, OC], F32, tag="acc")
            for hh in range(HH):
                nc.tensor.matmul(l_ps[:, 0:nn_], agT[:, hh, :], ow[:, hh, 0:nn_],
                                 start=(hh == 0), stop=False)
            nc.tensor.matmul(l_ps[:, 0:nn_], ones16_r[:], ob[:, 0:nn_],
                             start=False, stop=True)
            lsb = pool2.tile([B, OC], F32, tag="lsb")
            nc.scalar.copy(lsb[:, 0:nn_], l_ps[:, 0:nn_])
            nc.sync.dma_start(logits_d[:, nv:nv + nn_], lsb[:, 0:nn_])
            nv += nn_

    nc.finalize()
    return nc


_NC_CACHE = {}


def _get_nc():
    if "nc" not in _NC_CACHE:
        _NC_CACHE["nc"] = _build_nc()
    return _NC_CACHE["nc"]


def _host_prep(inputs):
    seq = np.asarray(inputs["seq"]).astype(np.int64)
    embed = np.asarray(inputs["embed"], dtype=np.float32)
    embedT = np.ascontiguousarray(embed.T)

    recip = np.ones(T, np.float32)
    s = np.arange(NC_)
    cnt = np.minimum(s + 1 + WINDOW, NC_) - s - 1
    recip[:NC_] = 1.0 / np.maximum(cnt, 1)
    recip[NC_ - 1] = 1.0  # empty window -> override slot uses hidden directly

    outw = np.asarray(inputs["out_w"], dtype=np.float32)
    outb = np.asarray(inputs["out_b"], dtype=np.float32)
    outw_pad = np.zeros((H, VS * NCORES), np.float32)
    outw_pad[:, :V] = outw
    outb_pad = np.zeros(VS * NCORES, np.float32)
    outb_pad[:V] = outb

    shared = {
        "ff1_w": np.asarray(inputs["ff1_w"], np.float32),
        "ff1_b": np.asarray(inputs["ff1_b"], np.float32),
        "ff2_w": np.asarray(inputs["ff2_w"], np.float32),
        "ff2_b": np.asarray(inputs["ff2_b"], np.float32),
        "ln_w": np.asarray(inputs["ln_w"], np.float32),
        "ln_b": np.asarray(inputs["ln_b"], np.float32),
        "fg_w": np.asarray(inputs["fg_w"], np.float32),
        "fg_b": np.asarray(inputs["fg_b"], np.float32).reshape(1),
        "rg1_w": np.asarray(inputs["rg1_w"], np.float32),
        "rg1_b": np.asarray(inputs["rg1_b"], np.float32),
        "rg2_w": np.asarray(inputs["rg2_w"], np.float32),
        "rg2_b": np.asarray(inputs["rg2_b"], np.float32).reshape(1),
        "q_w": np.asarray(inputs["q_w"], np.float32),
        "q_b": np.asarray(inputs["q_b"], np.float32),
        "recip": recip,
        "iota": np.arange(128, dtype=np.float32).reshape(128, 1),
        "ident": np.eye(128, dtype=np.float32),
    }
    in_maps = []
    for c in range(NCORES):
        m = dict(shared)
        h0T = np.empty((BL, H, T), np.float32)
        for b in range(BL):
            h0T[b] = embedT[:, seq[c * BL + b]]
        m["h0T"] = h0T
        m["out_w_sh"] = np.ascontiguousarray(outw_pad[:, c * VS:(c + 1) * VS])
        m["out_b_sh"] = np.ascontiguousarray(outb_pad[c * VS:(c + 1) * VS])
        in_maps.append(m)
    return in_maps


def kernel(**inputs):
    in_maps = _host_prep(inputs)
    nc = _get_nc()
    res = run_bass_kernel_spmd(nc, in_maps, list(range(NCORES)))
    out = np.concatenate([res.results[c]["logits"] for c in range(NCORES)], axis=1)
    return np.ascontiguousarray(out[:, :V])


# revision 19
# speedup vs baseline: 1.1604x; 1.1203x over previous
"""Trainium2 Bass kernel for nn_LookaheadModel (B=16, T=2048, H=512, V=50257).

Strategy (8 NeuronCores, SPMD):
- Data-parallel over batch: core c owns batches [2c, 2c+1] for the encoder /
  selection / attention pipeline.
- Tensor-parallel over vocab for the output projection: core c computes
  logits[:, c*VS:(c+1)*VS] for ALL 16 batches after an AllGather of the
  16 context vectors.
- The reference's topk/gather/pad logic is reformulated mask-wise (exactly
  equivalent: attention is permutation-invariant over memory slots and
  MEM_SLOTS == K + R exactly, so only the selected SET matters):
    * fwd top-512 / retro top-128 become per-batch score thresholds found by
      a 4-round 128-way histogram search on device (final step 6e-8 <<
      min boundary gap ~4e-5 for this model, host-validated).
    * attention runs over all 2045 candidates with -1e9 added to unselected
      positions (exp underflows to exactly 0, matching the reference).
- The embedding gather runs on the host (indirect DMA unavailable here);
  the device receives pre-gathered transposed embeddings h0T per core.
- Heavy matmuls use float32r (~1.4e-4 rel err; end-to-end impact ~8e-4,
  validated against the reference including selection-flip effects).

Layouts: h-major everywhere ([h partitions, t free]); zero transposes.
Windowed context sums are PSUM-accumulated identity matmuls with clamped
widths (the ragged tail falls out of partial-width accumulation).
"""
import numpy as np
from contextlib import ExitStack

import concourse.bass as bass
import concourse.bacc as bacc
import concourse.tile as tile
from concourse import mybir
from concourse.bass_utils import run_bass_kernel_spmd

F32 = mybir.dt.float32
F32R = mybir.dt.float32r
BF16 = mybir.dt.bfloat16
AL = mybir.AluOpType
AF = mybir.ActivationFunctionType
AX = mybir.AxisListType

B, T, H, V = 16, 2048, 512, 50257
NC_ = T - 3              # 2045 candidates
K_FWD, K_RETRO = 512, 128
WINDOW = 8
NCORES = 8
BL = B // NCORES         # batches per core (2)
VS = 6400                # vocab shard (25 uniform 256-wide tiles, 8*6400 >= V)
NCHUNK, CH = 4, 512      # encoder token chunks
EPS = 1e-5
HH = H // 128            # 4 h-tiles
JJ = 2 * H // 128        # 8 j-tiles
OC = 256                 # output-projection vocab chunk

SEARCH_LO = -16.0
SEARCH_STEPS = []
_step = 32.0 / 127.0
for _ in range(4):
    SEARCH_STEPS.append(_step)
    _step /= 126.0


def _build_nc():
    nc = bacc.Bacc(None, target_bir_lowering=False)
    D = lambda n, s, dt=F32: nc.declare_dram_parameter(n, s, dt, isOutput=False)

    h0T_d = D("h0T", [BL, H, T])
    ff1_d = D("ff1_w", [H, 2 * H]); ff1b_d = D("ff1_b", [2 * H])
    ff2_d = D("ff2_w", [2 * H, H]); ff2b_d = D("ff2_b", [H])
    lnw_d = D("ln_w", [H]); lnb_d = D("ln_b", [H])
    fg_d = D("fg_w", [H]); fgb_d = D("fg_b", [1])
    rg1_d = D("rg1_w", [2 * H, H]); rg1b_d = D("rg1_b", [H])
    rg2_d = D("rg2_w", [H]); rg2b_d = D("rg2_b", [1])
    qw_d = D("q_w", [H, H]); qb_d = D("q_b", [H])
    outw_d = D("out_w_sh", [VS // OC, 128, HH, OC]); outb_d = D("out_b_sh", [VS])
    recip_d = D("recip", [T])
    iota_d = D("iota", [128, 1])
    ident_d = D("ident", [128, 128])
    logits_d = nc.declare_dram_parameter("logits", [B, VS], F32, isOutput=True)

    with tile.TileContext(nc) as tc, ExitStack() as ctx:
        wpool = ctx.enter_context(tc.tile_pool(name="w", bufs=1))
        hpool = ctx.enter_context(tc.tile_pool(name="h", bufs=1))
        pool = ctx.enter_context(tc.tile_pool(name="p", bufs=1))
        pool2 = ctx.enter_context(tc.tile_pool(name="p2", bufs=2))
        rows = ctx.enter_context(tc.tile_pool(name="r", bufs=4))
        psumB = ctx.enter_context(tc.tile_pool(name="psB", bufs=1, space="PSUM"))
        psumA = ctx.enter_context(tc.tile_pool(name="psA", bufs=2, space="PSUM"))
        psumS = ctx.enter_context(tc.tile_pool(name="psS", bufs=2, space="PSUM"))
        dram = ctx.enter_context(tc.tile_pool(name="dr", bufs=1, space="DRAM"))

        def big_ps():
            return psumB.tile([128, T], F32, tag="big", name="bigps")

        # ---- persistent weights / constants ----
        ff1_sb = wpool.tile([128, HH, 2 * H], F32R)
        nc.sync.dma_start(ff1_sb[:], ff1_d[:, :].rearrange("(hh p) j -> p hh j", p=128).bitcast(F32R))
        ff2_sb = wpool.tile([128, JJ, H], F32R)
        nc.sync.dma_start(ff2_sb[:], ff2_d[:, :].rearrange("(jj p) h -> p jj h", p=128).bitcast(F32R))
        rg1_sb = wpool.tile([128, JJ, H], F32R)
        nc.sync.dma_start(rg1_sb[:], rg1_d[:, :].rearrange("(jj p) h -> p jj h", p=128).bitcast(F32R))
        ident_sb = wpool.tile([128, 128], F32R)
        nc.sync.dma_start(ident_sb[:], ident_d[:, :].bitcast(F32R))
        fg_sb = wpool.tile([128, HH], F32R)
        nc.sync.dma_start(fg_sb[:], fg_d[:].rearrange("(hh p) -> p hh", p=128).bitcast(F32R))
        rg2_sb = wpool.tile([128, HH], F32R)
        nc.sync.dma_start(rg2_sb[:], rg2_d[:].rearrange("(hh p) -> p hh", p=128).bitcast(F32R))
        ff1b_sb = wpool.tile([128, JJ], F32)
        nc.sync.dma_start(ff1b_sb[:], ff1b_d[:].rearrange("(jj p) -> p jj", p=128))
        ff2b_sb = wpool.tile([128, HH], F32)
        nc.sync.dma_start(ff2b_sb[:], ff2b_d[:].rearrange("(hh p) -> p hh", p=128))
        rg1b_sb = wpool.tile([128, HH], F32)
        nc.sync.dma_start(rg1b_sb[:], rg1b_d[:].rearrange("(hh p) -> p hh", p=128))
        qb_sb = wpool.tile([128, HH], F32)
        nc.sync.dma_start(qb_sb[:], qb_d[:].rearrange("(hh p) -> p hh", p=128))
        lnw_sb = wpool.tile([128, HH], F32)
        nc.sync.dma_start(lnw_sb[:], lnw_d[:].rearrange("(hh p) -> p hh", p=128))
        lnb_sb = wpool.tile([128, HH], F32)
        nc.sync.dma_start(lnb_sb[:], lnb_d[:].rearrange("(hh p) -> p hh", p=128))
        fgb_sb = wpool.tile([1, 1], F32)
        nc.sync.dma_start(fgb_sb[:], fgb_d[None, :])
        rg2b_sb = wpool.tile([1, 1], F32)
        nc.sync.dma_start(rg2b_sb[:], rg2b_d[None, :])
        iota_sb = wpool.tile([128, 1], F32)
        nc.sync.dma_start(iota_sb[:], iota_d[:])
        eps_sb = wpool.tile([1, 1], F32)
        nc.vector.memset(eps_sb[:], EPS)
        ones_row = wpool.tile([1, 128], F32)
        nc.vector.memset(ones_row[:], 1.0)
        ones_colf = wpool.tile([128, 1], F32)
        nc.vector.memset(ones_colf[:], 1.0)
        ones_colr = wpool.tile([128, 1], F32R)
        nc.scalar.copy(ones_colr[:], ones_colf[:])
        ones16_f = wpool.tile([1, B], F32)
        nc.vector.memset(ones16_f[:], 1.0)
        ones16_r = wpool.tile([1, B], F32R)
        nc.scalar.copy(ones16_r[:], ones16_f[:])
        ones_rowr = wpool.tile([1, 128], F32R)
        nc.scalar.copy(ones_rowr[:], ones_row[:])
        ctx_sb = wpool.tile([128, HH, BL], F32)
        zpad_f = wpool.tile([128, 32], F32)
        nc.vector.memset(zpad_f[:], 0.0)
        zpad_sb = wpool.tile([128, 32], F32R)
        nc.scalar.copy(zpad_sb[:], zpad_f[:])

        # recip replicated (fp32 exact broadcast)
        recip_row = rows.tile([1, T], F32, tag="row")
        nc.sync.dma_start(recip_row[:], recip_d[None, :])
        recrep_ps = big_ps()
        for q in range(T // 512):
            nc.tensor.matmul(recrep_ps[:, q * 512:(q + 1) * 512], ones_row[:],
                             recip_row[:, q * 512:(q + 1) * 512], start=True, stop=True)
        recip_rep = wpool.tile([128, T], F32)
        nc.scalar.copy(recip_rep[:], recrep_ps[:])

        def threshold_search(srow, k):
            """tau with count(srow > tau) == k. srow: [1, T] f32 sbuf row."""
            srep = pool.tile([128, T], F32, tag="srep")
            srep_ps = big_ps()
            for q in range(T // 512):
                nc.tensor.matmul(srep_ps[:, q * 512:(q + 1) * 512], ones_row[:],
                                 srow[:, q * 512:(q + 1) * 512], start=True, stop=True)
            nc.scalar.copy(srep[:], srep_ps[:])
            lo = None
            for r, step in enumerate(SEARCH_STEPS):
                tau_col = pool2.tile([128, 1], F32, tag="tcol")
                if r == 0:
                    nc.vector.tensor_scalar(tau_col[:], iota_sb[:], step, SEARCH_LO,
                                            AL.mult, AL.add)
                else:
                    lo_ps = psumS.tile([128, 1], F32, tag="small")
                    nc.tensor.matmul(lo_ps[:], ones_row[:], lo[:], start=True, stop=True)
                    nc.vector.scalar_tensor_tensor(tau_col[:], iota_sb[:], step,
                                                   lo_ps[:], AL.mult, AL.add)
                ntau = pool2.tile([128, 1], F32, tag="ntau")
                nc.vector.tensor_scalar_mul(ntau[:], tau_col[:], -1.0)
                sgn = pool.tile([128, T], BF16, tag="sgn")
                scnt = pool2.tile([128, 1], F32, tag="scnt")
                nc.scalar.activation(sgn[:], srep[:], AF.Sign, bias=ntau[:],
                                     scale=1.0, accum_out=scnt[:])
                g = pool2.tile([128, 1], F32, tag="g")
                nc.vector.tensor_scalar(g[:], scnt[:], float(2 * k - T) - 0.5, None,
                                        AL.is_ge)
                cnt_ps = psumS.tile([1, 1], F32, tag="small")
                nc.tensor.matmul(cnt_ps[:], g[:], ones_colf[:],
                                 start=True, stop=True)
                nlo = pool2.tile([1, 1], F32, tag="nlo")
                nc.vector.tensor_scalar(nlo[:], cnt_ps[:], -1.0, step, AL.add, AL.mult)
                if r == 0:
                    nc.vector.tensor_scalar_add(nlo[:], nlo[:], SEARCH_LO)
                else:
                    nc.vector.tensor_scalar_add(nlo[:], nlo[:], lo[:, 0:1])
                lo = nlo
            return lo

        for b in range(BL):
            hid = hpool.tile([128, HH, T], F32R, tag="hid")

            def enc_matmul_phase(c):
                t0 = c * CH
                h0 = pool2.tile([128, HH, CH], F32R, tag="h0hsb", name="h0")
                nc.sync.dma_start(
                    h0[:], h0T_d[b, :, t0:t0 + CH].rearrange("(hh p) t -> p hh t", p=128).bitcast(F32R))
                hT_ps = big_ps().rearrange("p (hh t) -> p hh t", hh=HH)
                for jj in range(JJ):
                    y1_ps = psumS.tile([128, CH], F32, tag="small", name="y1ps")
                    for hh in range(HH):
                        nc.tensor.matmul(y1_ps[:],
                                         ff1_sb[:, hh, jj * 128:(jj + 1) * 128],
                                         h0[:, hh, :],
                                         start=(hh == 0), stop=(hh == HH - 1))
                    y1 = pool2.tile([128, CH], F32R, tag="y1", name="y1")
                    nc.scalar.activation(y1[:], y1_ps[:], AF.Relu,
                                         bias=ff1b_sb[:, jj:jj + 1], scale=1.0)
                    for hh in range(HH):
                        nc.tensor.matmul(hT_ps[:, hh, :],
                                         ff2_sb[:, jj, hh * 128:(hh + 1) * 128],
                                         y1[:],
                                         start=(jj == 0), stop=False)
                for hh in range(HH):  # residual
                    nc.tensor.matmul(hT_ps[:, hh, :], ident_sb[:], h0[:, hh, :],
                                     start=False, stop=True)
                return hT_ps

            def enc_ln_phase(c, hT_ps):
                t0 = c * CH
                hsb = pool2.tile([128, HH, CH], F32R, tag="h0hsb", name="hsb")
                mu_ps = psumA.tile([1, CH], F32, tag="acc", name="mups")
                s2_ps = psumA.tile([1, CH], F32, tag="acc", name="s2ps")
                for hh in range(HH):
                    nc.scalar.activation(hsb[:, hh, :], hT_ps[:, hh, :], AF.Identity,
                                         bias=ff2b_sb[:, hh:hh + 1], scale=1.0)
                    sq = pool2.tile([128, CH], F32R, tag="tmp512", name="sq")
                    nc.scalar.activation(sq[:], hsb[:, hh, :], AF.Square)
                    nc.tensor.matmul(mu_ps[:], ones_colr[:], hsb[:, hh, :],
                                     start=(hh == 0), stop=(hh == HH - 1))
                    nc.tensor.matmul(s2_ps[:], ones_colr[:], sq[:],
                                     start=(hh == 0), stop=(hh == HH - 1))
                negmu = pool.tile([1, CH], F32R, tag="negmu", name="negmu")
                nc.vector.tensor_scalar_mul(negmu[:], mu_ps[:], -1.0 / H)
                mu2 = pool.tile([1, CH], F32, tag="mu2", name="mu2")
                nc.scalar.activation(mu2[:], negmu[:], AF.Square)
                var_ps = psumS.tile([1, CH], F32, tag="small", name="varps")
                nc.vector.scalar_tensor_tensor(var_ps[:], s2_ps[:], 1.0 / H, mu2[:],
                                               AL.mult, AL.subtract)
                # rstd = exp(-0.5 * ln(var + eps))
                lnv = pool.tile([1, CH], F32, tag="lnv", name="lnv")
                nc.scalar.activation(lnv[:], var_ps[:], AF.Ln, bias=eps_sb[:], scale=1.0)
                rstd = pool.tile([1, CH], F32R, tag="rstd", name="rstd")
                nc.scalar.activation(rstd[:], lnv[:], AF.Exp, scale=-0.5)
                nm_ps = psumS.tile([128, CH], F32, tag="small", name="nmps")
                rs_ps = psumS.tile([128, CH], F32, tag="small", name="rsps")
                nc.tensor.matmul(nm_ps[:], ones_rowr[:], negmu[:], start=True, stop=True)
                nc.tensor.matmul(rs_ps[:], ones_rowr[:], rstd[:], start=True, stop=True)
                for hh in range(HH):
                    t1 = pool2.tile([128, CH], F32, tag="tmp512", name="t1")
                    nc.vector.tensor_add(t1[:], hsb[:, hh, :], nm_ps[:])
                    nc.vector.tensor_mul(t1[:], t1[:], rs_ps[:])
                    nc.vector.tensor_scalar(hid[:, hh, t0:t0 + CH], t1[:],
                                            lnw_sb[:, hh:hh + 1], lnb_sb[:, hh:hh + 1],
                                            AL.mult, AL.add)

            def fwd_score_phase(c, fwd_row):
                t0 = c * CH
                f_ps = psumA.tile([1, CH], F32, tag="acc", name="fps")
                for hh in range(HH):
                    nc.tensor.matmul(f_ps[:], fg_sb[:, hh:hh + 1], hid[:, hh, t0:t0 + CH],
                                     start=(hh == 0), stop=(hh == HH - 1))
                nc.scalar.activation(fwd_row[:, t0:t0 + CH], f_ps[:], AF.Identity,
                                     bias=fgb_sb[:], scale=1.0)

            def make_htail():
                htail = pool.tile([128, HH, CH + 16], F32R, tag="htail", name="htail")
                nvt = NC_ - (NCHUNK - 1) * CH    # 509: hid[1536:2045] are summable
                nc.vector.tensor_copy(htail[:, :, 0:nvt],
                                      hid[:, :, (NCHUNK - 1) * CH:NC_])
                for hh in range(HH):
                    nc.vector.tensor_copy(htail[:, hh, nvt:CH + 16],
                                          zpad_sb[:, 0:CH + 16 - nvt])
                return htail

            def retro_phase(c, retro_row, htail):
                t0 = c * CH
                last = c == NCHUNK - 1
                ctxw = pool.tile([128, HH, CH], F32R, tag="ctxw", name="ctxw")
                for hh in range(HH):
                    src_ = htail[:, hh, :] if last else hid[:, hh, t0:t0 + CH + 16]
                    w2 = pool.tile([128, CH + 10], F32, tag="w2", name="w2")
                    nc.vector.tensor_add(w2[:], src_[:, 1:CH + 11], src_[:, 2:CH + 12])
                    w4 = pool.tile([128, CH + 6], F32, tag="w4", name="w4")
                    nc.vector.tensor_add(w4[:], w2[:, 0:CH + 6], w2[:, 2:CH + 8])
                    seg = pool2.tile([128, CH], F32, tag="tmp512", name="seg")
                    nc.vector.tensor_add(seg[:], w4[:, 0:CH], w4[:, 4:CH + 4])
                    nc.vector.tensor_mul(ctxw[:, hh, :], seg[:],
                                         recip_rep[:, t0:t0 + CH])
                if last:
                    # position 2044: empty window -> ctx = hidden[2044]
                    nc.vector.tensor_copy(ctxw[:, :, NC_ - 1 - t0:NC_ - t0],
                                          hid[:, :, NC_ - 1:NC_])
                r_ps = psumA.tile([1, CH], F32, tag="acc", name="rps")
                for hh in range(HH):
                    z_ps = psumS.tile([128, CH], F32, tag="small", name="zps")
                    for kk in range(JJ):
                        rhs = hid[:, kk, t0:t0 + CH] if kk < HH else ctxw[:, kk - HH, :]
                        nc.tensor.matmul(z_ps[:],
                                         rg1_sb[:, kk, hh * 128:(hh + 1) * 128],
                                         rhs, start=(kk == 0), stop=(kk == JJ - 1))
                    z1 = pool2.tile([128, CH], F32R, tag="z1", name="z1")
                    nc.scalar.activation(z1[:], z_ps[:], AF.Relu,
                                         bias=rg1b_sb[:, hh:hh + 1], scale=1.0)
                    nc.tensor.matmul(r_ps[:], rg2_sb[:, hh:hh + 1], z1[:],
                                     start=(hh == 0), stop=(hh == HH - 1))
                nc.scalar.activation(retro_row[:, t0:t0 + CH], r_ps[:], AF.Identity,
                                     bias=rg2b_sb[:], scale=1.0)

            # ---- software-pipelined emission ----
            fwd_row = rows.tile([1, T], F32, tag="row")
            retro_row = rows.tile([1, T], F32, tag="row")
            hT0 = enc_matmul_phase(0)
            hT1 = enc_matmul_phase(1)
            enc_ln_phase(0, hT0)
            hT2 = enc_matmul_phase(2)
            enc_ln_phase(1, hT1)
            fwd_score_phase(0, fwd_row)
            hT3 = enc_matmul_phase(3)
            enc_ln_phase(2, hT2)
            fwd_score_phase(1, fwd_row)
            retro_phase(0, retro_row, None)
            enc_ln_phase(3, hT3)
            fwd_score_phase(2, fwd_row)
            fwd_score_phase(3, fwd_row)
            nc.vector.memset(fwd_row[:, NC_:T], -1.0e30)
            # q + attention scores (independent of masks; overlap searches)
            q_ps = psumA.tile([128, HH, 2], F32, tag="acc")
            for mm in range(HH):
                qws = pool2.tile([128, HH, 128], F32R, tag="qws", name="qws")
                nc.sync.dma_start(
                    qws[:], qw_d[:, mm * 128:(mm + 1) * 128].rearrange("(hh p) j -> p hh j", p=128).bitcast(F32R))
                for hh in range(HH):
                    nc.tensor.matmul(q_ps[:, mm, :],
                                     qws[:, hh, :],
                                     hid[:, hh, T - 2:T],
                                     start=(hh == 0), stop=(hh == HH - 1))
            q_sb = pool2.tile([128, HH], F32R, tag="qsb")
            for mm in range(HH):
                nc.scalar.activation(q_sb[:, mm:mm + 1], q_ps[:, mm, 0:1],
                                     AF.Identity, bias=qb_sb[:, mm:mm + 1], scale=1.0)
            att_row = rows.tile([1, T], F32, tag="row")
            for c in range(NCHUNK):
                t0 = c * CH
                a_ps = psumA.tile([1, CH], F32, tag="acc", name="aps")
                for hh in range(HH):
                    nc.tensor.matmul(a_ps[:], q_sb[:, hh:hh + 1], hid[:, hh, t0:t0 + CH],
                                     start=(hh == 0), stop=(hh == HH - 1))
                nc.scalar.copy(att_row[:, t0:t0 + CH], a_ps[:])

            retro_phase(1, retro_row, None)
            tau_f = threshold_search(fwd_row, K_FWD)
            retro_phase(2, retro_row, None)
            htail = make_htail()
            retro_phase(3, retro_row, htail)
            nc.vector.memset(retro_row[:, NC_:T], -1.0e30)

            # ================= masks =================
            mf = rows.tile([1, T], F32, tag="row")
            nc.vector.tensor_scalar(mf[:], fwd_row[:], tau_f[:], None, AL.is_gt)
            rmask_row = rows.tile([1, T], F32, tag="row")
            nc.vector.scalar_tensor_tensor(rmask_row[:], mf[:], -1.0e30, retro_row[:],
                                           AL.mult, AL.add)
            tau_r = threshold_search(rmask_row, K_RETRO)
            mr = rows.tile([1, T], F32, tag="row")
            nc.vector.tensor_scalar(mr[:], rmask_row[:], tau_r[:], None, AL.is_gt)
            # mf, mr disjoint -> pen = mf + mr - 1 in {-1, 0}
            pen = rows.tile([1, T], F32, tag="row")
            nc.vector.scalar_tensor_tensor(pen[:], mf[:], -1.0, mr[:], AL.add, AL.add)

            # att += pen * 1e9  (in-place elementwise on DVE)
            nc.vector.scalar_tensor_tensor(att_row[:], pen[:], 1.0e9, att_row[:],
                                           AL.mult, AL.add)

            amax = pool2.tile([1, 1], F32, tag="amax")
            nc.vector.tensor_reduce(amax[:], att_row[:], AX.X, AL.max)
            namax = pool2.tile([1, 1], F32, tag="namax")
            nc.vector.tensor_scalar_mul(namax[:], amax[:], -1.0)
            esum = pool2.tile([1, 1], F32, tag="esum")
            attn_row = rows.tile([1, T], F32R, tag="row")
            nc.scalar.activation(attn_row[:], att_row[:], AF.Exp, bias=namax[:],
                                 scale=1.0, accum_out=esum[:])
            rsum = pool2.tile([1, 1], F32, tag="rsum")
            nc.vector.reciprocal(rsum[:], esum[:])

            # ctx = (sum_t e[t] * hid[:, t]) * (1/esum)
            at_ps = big_ps()
            for q in range(T // 512):
                nc.tensor.matmul(at_ps[:, q * 512:(q + 1) * 512], ones_rowr[:],
                                 attn_row[:, q * 512:(q + 1) * 512], start=True, stop=True)
            rs_col_ps = psumS.tile([128, 1], F32, tag="small")
            nc.tensor.matmul(rs_col_ps[:], ones_row[:], rsum[:], start=True, stop=True)
            rs_col = pool2.tile([128, 1], F32, tag="rscols")
            nc.scalar.copy(rs_col[:], rs_col_ps[:])
            for hh in range(HH):
                craw = pool2.tile([128, 4], F32, tag="craw")
                for q in range(T // 512):
                    prod = pool2.tile([128, CH], F32, tag="tmp512", name="prod")
                    nc.vector.tensor_mul(prod[:], hid[:, hh, q * 512:(q + 1) * 512],
                                         at_ps[:, q * 512:(q + 1) * 512])
                    nc.vector.tensor_reduce(craw[:, q:q + 1], prod[:], AX.X, AL.add)
                craw2 = pool2.tile([128, 1], F32, tag="craw2")
                nc.vector.tensor_reduce(craw2[:], craw[:], AX.X, AL.add)
                nc.vector.tensor_scalar(ctx_sb[:, hh, b:b + 1], craw2[:], rs_col[:],
                                        None, AL.mult)

        # ================= allgather + output projection =================
        ctxl_d = dram.tile([BL, H], F32)
        for b in range(BL):
            nc.sync.dma_start(ctxl_d[b].rearrange("(hh p) -> p hh", p=128),
                              ctx_sb[:, :, b:b + 1].rearrange("p hh o -> p (hh o)"))
        ag_d = dram.tile([B, H], F32)
        nc.gpsimd.collective_compute(
            "AllGather", AL.bypass,
            replica_groups=[list(range(NCORES))],
            ins=[ctxl_d.opt()], outs=[ag_d.opt()])
        agT = wpool.tile([128, HH, B], F32R)
        for hh in range(HH):
            nc.sync.dma_start(
                agT[:, hh, :],
                ag_d[:, hh * 128:(hh + 1) * 128].rearrange("b p -> p b").bitcast(F32R))

        for t in range(VS // OC):
            nv = t * OC
            ow = pool.tile([128, HH, OC], F32R, tag="ow", bufs=2, name="ow")
            nc.sync.dma_start(ow[:], outw_d[t].bitcast(F32R))
            ob = pool2.tile([1, OC], F32R, tag="ob")
            nc.sync.dma_start(ob[:], outb_d[None, nv:nv + OC].bitcast(F32R))
            l_ps = psumA.tile([B, OC], F32, tag="acc")
            for hh in range(HH):
                nc.tensor.matmul(l_ps[:], agT[:, hh, :], ow[:, hh, :],
                                 start=(hh == 0), stop=False)
            nc.tensor.matmul(l_ps[:], ones16_r[:], ob[:],
                             start=False, stop=True)
            lsb = pool2.tile([B, OC], F32, tag="lsb")
            nc.scalar.copy(lsb[:], l_ps[:])
            nc.sync.dma_start(logits_d[:, nv:nv + OC], lsb[:])

    nc.finalize()
    return nc


_NC_CACHE = {}


def _get_nc():
    if "nc" not in _NC_CACHE:
        _NC_CACHE["nc"] = _build_nc()
    return _NC_CACHE["nc"]


def _host_prep(inputs):
    seq = np.asarray(inputs["seq"]).astype(np.int64)
    embed = np.asarray(inputs["embed"], dtype=np.float32)
    embedT = np.ascontiguousarray(embed.T)

    recip = np.ones(T, np.float32)
    s = np.arange(NC_)
    cnt = np.minimum(s + 1 + WINDOW, NC_) - s - 1
    recip[:NC_] = 1.0 / np.maximum(cnt, 1)
    recip[NC_ - 1] = 1.0  # empty window -> override slot uses hidden directly

    outw = np.asarray(inputs["out_w"], dtype=np.float32)
    outb = np.asarray(inputs["out_b"], dtype=np.float32)
    outw_pad = np.zeros((H, VS * NCORES), np.float32)
    outw_pad[:, :V] = outw
    outb_pad = np.zeros(VS * NCORES, np.float32)
    outb_pad[:V] = outb
    # [H, VS*NCORES] -> per core [NT, 128, HH, OC] tiled layout
    NT = VS // OC
    outw_tiled = np.ascontiguousarray(
        outw_pad.reshape(HH, 128, NCORES, NT, OC).transpose(2, 3, 1, 0, 4))

    shared = {
        "ff1_w": np.asarray(inputs["ff1_w"], np.float32),
        "ff1_b": np.asarray(inputs["ff1_b"], np.float32),
        "ff2_w": np.asarray(inputs["ff2_w"], np.float32),
        "ff2_b": np.asarray(inputs["ff2_b"], np.float32),
        "ln_w": np.asarray(inputs["ln_w"], np.float32),
        "ln_b": np.asarray(inputs["ln_b"], np.float32),
        "fg_w": np.asarray(inputs["fg_w"], np.float32),
        "fg_b": np.asarray(inputs["fg_b"], np.float32).reshape(1),
        "rg1_w": np.asarray(inputs["rg1_w"], np.float32),
        "rg1_b": np.asarray(inputs["rg1_b"], np.float32),
        "rg2_w": np.asarray(inputs["rg2_w"], np.float32),
        "rg2_b": np.asarray(inputs["rg2_b"], np.float32).reshape(1),
        "q_w": np.asarray(inputs["q_w"], np.float32),
        "q_b": np.asarray(inputs["q_b"], np.float32),
        "recip": recip,
        "iota": np.arange(128, dtype=np.float32).reshape(128, 1),
        "ident": np.eye(128, dtype=np.float32),
    }
    in_maps = []
    for c in range(NCORES):
        m = dict(shared)
        h0T = np.empty((BL, H, T), np.float32)
        for b in range(BL):
            h0T[b] = embedT[:, seq[c * BL + b]]
        m["h0T"] = h0T
        m["out_w_sh"] = outw_tiled[c]
        m["out_b_sh"] = np.ascontiguousarray(outb_pad[c * VS:(c + 1) * VS])
        in_maps.append(m)
    return in_maps


def kernel(**inputs):
    in_maps = _host_prep(inputs)
    nc = _get_nc()
    res = run_bass_kernel_spmd(nc, in_maps, list(range(NCORES)))
    out = np.concatenate([res.results[c]["logits"] for c in range(NCORES)], axis=1)
    return np.ascontiguousarray(out[:, :V])
